# revision 12
# baseline (speedup 1.0000x reference)
"""Trainium2 Bass kernel for nn_BasicIdentifier: fused per-tag masked
weighted sum-pool + per-tag linear + log-sigmoid head + BCE loss.

Math (per batch b):
    pooled[t, d] = sum_s maskf[b,s] * samples[b,s,t] * emb[b,s,d]
    logits[t]    = sum_d pooled[t,d] * W[t,d] + bias[t]
    log_probs    = log_sigmoid(logits) = -softplus(-logits)
    preds        = (log_probs > LOG_THRESH) as int32
    loss         = mean_b mean_t [ softplus(logits) - soft[t]*logits ],
                   soft = 0.1 + 0.8*labels[:, :9]

Mapping: pooled is computed on the PE as ws_chunk.T @ emb_chunk where
ws[s, t] = maskf[s]*samples[s,t] is the stationary operand [128, 9] and emb
streams through in its natural [s, d] DRAM layout (no transposes anywhere).
Contraction (s) is the partition dim; 32 accumulating matmuls per batch.

Sharding: data-parallel over batch, B=16 over 8 cores -> 2 batches/core.
W/b replicated. Loss: each core emits its additive contribution (already
scaled by 1/(B*T)); the host sums 8 scalars while gathering.
"""

import numpy as np

import concourse.bass as bass
import concourse.bacc as bacc
import concourse.mybir as mybir
import concourse.tile as tile
from concourse.bass_utils import run_bass_kernel_spmd

F32 = mybir.dt.float32
F32R = mybir.dt.float32r
I32 = mybir.dt.int32

B, S, D, T = 16, 4096, 512, 9
U = T + 1  # samples last-dim (10); column T is unused
N_CORES = 8
BPC = B // N_CORES  # batches per core
LOG_THRESH = float(np.log(0.5 + 1e-05))
SMOOTHING = 0.1

P = 128           # SBUF partitions
JCH = S // P      # 32 contraction chunks; s = p*JCH + j
N_DMA = 8           # emb DMAs per batch
JPD = JCH // N_DMA  # j-chunks per DMA (4) -> [128, JPD*D] = 1 MiB per DMA
EMB_BUFS = 5

_MODULE_CACHE = {}


def _build_module(repeats: int = 1) -> bass.Bass:
    """repeats > 1 wraps the whole computation in an on-device For_i loop;
    used only by the timing harness (outputs are idempotent per iteration)."""
    nc = bacc.Bacc("TRN2")

    emb = nc.declare_dram_parameter("emb_msg", [BPC, S, D], F32R, isOutput=False)
    mask = nc.declare_dram_parameter("mask", [BPC, S], I32, isOutput=False)
    samples = nc.declare_dram_parameter("samples", [BPC, S, U], F32, isOutput=False)
    labels = nc.declare_dram_parameter("labels", [BPC, U], I32, isOutput=False)
    w_d = nc.declare_dram_parameter("W", [T, D], F32, isOutput=False)
    b_d = nc.declare_dram_parameter("b", [T], F32, isOutput=False)

    logits_o = nc.declare_dram_parameter("logits", [BPC, T], F32, isOutput=True)
    lp_o = nc.declare_dram_parameter("log_probs", [BPC, T], F32, isOutput=True)
    preds_o = nc.declare_dram_parameter("preds", [BPC, T], I32, isOutput=True)
    loss_o = nc.declare_dram_parameter("loss", [1, 1], F32, isOutput=True)

    with tile.TileContext(nc) as tc:
        with (
            tc.tile_pool(name="consts", bufs=1) as consts,
            tc.tile_pool(name="emb", bufs=EMB_BUFS) as emb_pool,
            tc.tile_pool(name="prep", bufs=2) as prep,
            tc.tile_pool(name="ep", bufs=2) as ep,
            tc.tile_pool(name="psum", bufs=2, space="PSUM") as psum,
            tc.tile_pool(name="psum_loss", bufs=1, space="PSUM") as psum_loss,
        ):
            # --- replicated constants ---
            w_t = consts.tile([T, D], F32, tag="w")
            nc.gpsimd.dma_start(out=w_t[:, :], in_=w_d[:, :])
            b_t = consts.tile([T, 1], F32, tag="b")
            nc.gpsimd.dma_start(out=b_t[:, :], in_=b_d.rearrange("(t o) -> t o", o=1))
            ones_t = consts.tile([T, 1], F32, tag="ones")
            nc.vector.memset(ones_t[:, :], 1.0)
            # negated per-tag loss accumulator, one column per local batch
            negloss = consts.tile([T, BPC], F32, tag="negloss")

            import contextlib
            rep_ctx = (
                tc.For_i(0, repeats, 1) if repeats > 1 else contextlib.nullcontext()
            )
            with rep_ctx:
                for lb in range(BPC):
                    # --- per-batch small inputs: mask, samples, labels ---
                    mask_i = prep.tile([P, JCH], I32, tag="mask_i")
                    nc.gpsimd.dma_start(
                        out=mask_i[:, :],
                        in_=mask[lb].rearrange("(p j) -> p j", j=JCH),
                    )
                    smp = prep.tile([P, JCH * U], F32, tag="smp")
                    nc.gpsimd.dma_start(
                        out=smp[:, :].rearrange("p (j u) -> p j u", u=U),
                        in_=samples[lb].rearrange("(p j) u -> p j u", j=JCH),
                    )
                    lab_i = prep.tile([T, 1], I32, tag="lab_i")
                    nc.gpsimd.dma_start(
                        out=lab_i[:, :],
                        in_=labels[lb].rearrange("(u o) -> u o", o=1)[0:T],
                    )

                    maskf = prep.tile([P, JCH], F32, tag="maskf")
                    nc.vector.tensor_copy(maskf[:, :], mask_i[:, :])

                    # ws[p, j*U + u] = maskf[p, j] * samples[s(p,j), u]
                    # One tensor_scalar per j-chunk: the per-partition scalar AP
                    # maskf[:, j] broadcasts over the U samples columns. (A single
                    # fused op needs 3D operand APs, which force the S3S3D3_TT
                    # encoding with no room for sync waits -> walrus rejects.)
                    ws = prep.tile([P, JCH * U], F32R, tag="ws")
                    for j in range(JCH):
                        nc.vector.tensor_scalar_mul(
                            ws[:, j * U : (j + 1) * U],
                            smp[:, j * U : (j + 1) * U],
                            maskf[:, j : j + 1],
                        )

                    labf = ep.tile([T, 1], F32, tag="labf")
                    nc.vector.tensor_copy(labf[:, :], lab_i[:, :])
                    soft = ep.tile([T, 1], F32, tag="soft")
                    nc.vector.tensor_scalar(
                        soft[:, :], labf[:, :], 1.0 - 2.0 * SMOOTHING, SMOOTHING,
                        op0=mybir.AluOpType.mult, op1=mybir.AluOpType.add,
                    )
                    # --- pooled[t, d] accumulated over 32 chunk-matmuls ---
                    pooled = psum.tile([T, D], F32, tag="pooled")
                    emb_b = emb[lb].rearrange("(p j) d -> p j d", j=JCH)
                    for dj in range(N_DMA):
                        et = emb_pool.tile([P, JPD * D], F32R, tag="et")
                        # alternate the two HWDGE rings (SP / ACT sequencer)
                        dma_eng = nc.sync if dj % 2 == 0 else nc.scalar
                        dma_eng.dma_start(
                            out=et[:, :].rearrange("p (j d) -> p j d", d=D),
                            in_=emb_b[:, dj * JPD : (dj + 1) * JPD, :],
                        )
                        for j8 in range(JPD):
                            j = dj * JPD + j8
                            nc.tensor.matmul(
                                pooled[:, :],
                                lhsT=ws[:, j * U : j * U + T],
                                rhs=et[:, j8 * D : (j8 + 1) * D],
                                start=(j == 0),
                                stop=(j == JCH - 1),
                            )

                    # --- epilogue: logits, log_probs, preds, loss terms ---
                    # logits = b + sum_d pooled * W, decomposed into three DVE
                    # ops (the fused InstTensorTensorReduce crashes the exec
                    # unit on this runtime, micro-verified in isolation)
                    scr = ep.tile([T, D], F32, tag="scr")
                    nc.vector.tensor_tensor(
                        scr[:, :], pooled[:, :], w_t[:, :], op=mybir.AluOpType.mult
                    )
                    lg0 = ep.tile([T, 1], F32, tag="lg0")
                    nc.vector.reduce_sum(lg0[:, :], scr[:, :], axis=mybir.AxisListType.X)
                    logits_t = ep.tile([T, 1], F32, tag="logits")
                    nc.vector.tensor_add(logits_t[:, :], lg0[:, :], b_t[:, :])

                    # softplus via the stable identity
                    #   softplus(+-x) = log1p(exp(-|x|)) + relu(+-x)
                    # (the compiler's lower_act has no Softplus set; Exp/Ln do)
                    rp = ep.tile([T, 1], F32, tag="rp")
                    nc.vector.tensor_scalar_max(rp[:, :], logits_t[:, :], 0.0)
                    rn = ep.tile([T, 1], F32, tag="rn")
                    nc.vector.tensor_scalar(
                        rn[:, :], logits_t[:, :], -1.0, 0.0,
                        op0=mybir.AluOpType.mult, op1=mybir.AluOpType.max,
                    )
                    absx = ep.tile([T, 1], F32, tag="absx")
                    nc.vector.tensor_add(absx[:, :], rp[:, :], rn[:, :])
                    ex = ep.tile([T, 1], F32, tag="ex")
                    nc.scalar.activation(
                        ex[:, :], absx[:, :],
                        mybir.ActivationFunctionType.Exp, scale=-1.0,
                    )  # exp(-|x|)
                    l1p = ep.tile([T, 1], F32, tag="l1p")
                    nc.scalar.activation(
                        l1p[:, :], ex[:, :],
                        mybir.ActivationFunctionType.Ln, bias=1.0,
                    )  # log(1 + exp(-|x|))
                    sp = ep.tile([T, 1], F32, tag="sp")
                    nc.vector.tensor_add(sp[:, :], l1p[:, :], rp[:, :])
                    spn = ep.tile([T, 1], F32, tag="spn")
                    nc.vector.tensor_add(spn[:, :], l1p[:, :], rn[:, :])

                    lp_t = ep.tile([T, 1], F32, tag="lp")
                    nc.vector.tensor_scalar_mul(lp_t[:, :], spn[:, :], -1.0)

                    preds_t = ep.tile([T, 1], I32, tag="preds")
                    nc.vector.tensor_scalar(
                        preds_t[:, :], spn[:, :], -LOG_THRESH, None,
                        op0=mybir.AluOpType.is_lt,
                    )

                    # negloss[:, lb] = soft*logits - softplus(logits) = -loss_t
                    nc.vector.scalar_tensor_tensor(
                        out=negloss[:, lb : lb + 1],
                        in0=logits_t[:, :],
                        scalar=soft[:, :],
                        in1=sp[:, :],
                        op0=mybir.AluOpType.mult,
                        op1=mybir.AluOpType.subtract,
                    )

                    # --- per-batch outputs ---
                    nc.sync.dma_start(
                        out=logits_o[lb].rearrange("(t o) -> t o", o=1),
                        in_=logits_t[:, :],
                    )
                    nc.scalar.dma_start(
                        out=lp_o[lb].rearrange("(t o) -> t o", o=1), in_=lp_t[:, :]
                    )
                    nc.gpsimd.dma_start(
                        out=preds_o[lb].rearrange("(t o) -> t o", o=1),
                        in_=preds_t[:, :],
                    )

                # --- loss: reduce negloss [T, BPC] over tags (PE) then batches ---
                loss_ps = psum_loss.tile([1, BPC], F32, tag="loss_ps")
                nc.tensor.matmul(
                    loss_ps[:, :], lhsT=ones_t[:, :], rhs=negloss[:, :],
                    start=True, stop=True,
                )
                loss_scr = ep.tile([1, BPC], F32, tag="loss_scr")
                loss_sb = ep.tile([1, 1], F32, tag="loss_sb")
                # scale by -1/(B*T) (undo negation + global mean) and reduce
                nc.scalar.activation(
                    loss_scr[:, :], loss_ps[:, :],
                    mybir.ActivationFunctionType.Copy,
                    bias=0.0, scale=-1.0 / (B * T),
                    accum_out=loss_sb[:, :],
                )
                nc.sync.dma_start(out=loss_o[:, :], in_=loss_sb[:, :])

    nc.finalize()
    return nc


def get_module(repeats: int = 1) -> bass.Bass:
    key = ("nc", repeats)
    if key not in _MODULE_CACHE:
        _MODULE_CACHE[key] = _build_module(repeats)
    return _MODULE_CACHE[key]


def make_in_maps(emb_msg, mask, samples, labels, W, b):
    emb_msg = np.asarray(emb_msg, dtype=np.float32)
    mask = np.asarray(mask, dtype=np.int32)
    samples = np.asarray(samples, dtype=np.float32)
    labels = np.asarray(labels, dtype=np.int32)
    W = np.ascontiguousarray(np.asarray(W, dtype=np.float32))
    b = np.ascontiguousarray(np.asarray(b, dtype=np.float32))
    in_maps = []
    for c in range(N_CORES):
        sl = slice(c * BPC, (c + 1) * BPC)
        in_maps.append(
            {
                "emb_msg": np.ascontiguousarray(emb_msg[sl]),
                "mask": np.ascontiguousarray(mask[sl]),
                "samples": np.ascontiguousarray(samples[sl]),
                "labels": np.ascontiguousarray(labels[sl]),
                "W": W,
                "b": b,
            }
        )
    return in_maps


def gather_outputs(results):
    all_logits = np.concatenate([r["logits"] for r in results], axis=0)
    log_probs = np.concatenate([r["log_probs"] for r in results], axis=0)
    preds = np.concatenate([r["preds"] for r in results], axis=0)
    loss = np.float32(sum(float(r["loss"][0, 0]) for r in results))
    return all_logits, log_probs, preds, loss


def run(emb_msg, mask, samples, labels, W, b, **run_kwargs):
    nc = get_module()
    in_maps = make_in_maps(emb_msg, mask, samples, labels, W, b)
    res = run_bass_kernel_spmd(nc, in_maps, core_ids=list(range(N_CORES)), **run_kwargs)
    return gather_outputs(res.results), res


def kernel(emb_msg, mask, samples, labels, W, b):
    outputs, _ = run(emb_msg, mask, samples, labels, W, b)
    return outputs

